# revision 7
# baseline (speedup 1.0000x reference)
"""Trainium2 Bass kernel for nn_DecoderAttentionRNN.

Data-parallel over the N_cases axis: 8 NeuronCores, 8 cases each.
Per-core everything lives in SBUF; the decode steps are fully unrolled.
Activations are kept transposed ([feature-partition, case-free]) so
DVE/ACT ops run wide; weights are pre-transposed on the host into the
stationary [K, M] layouts the PE wants.

Host-side numpy does ONLY data marshalling (sharding, transposes,
dtype casts, embedding row gather, bias folding) - every model FLOP
(matmuls, tanh/sigmoid/exp, softmax, weighted sums) runs on device.
"""

import contextlib
import os
import numpy as np

import concourse.bass as bass
import concourse.tile as tile
from concourse import mybir
from concourse.bass_utils import run_bass_kernel_spmd

# Model dims (fixed by the problem)
L, T, N, H, E, A, V, S = 2, 128, 64, 512, 256, 200, 1000, 40
SOS_ID = 1

NCORES = 8
NSH = N // NCORES          # 8 cases per core
HC = H // 128              # 4 h-chunks
EC = E // 128              # 2 e-chunks
G3 = 3 * H                 # 1536 gate features
GC = G3 // 128             # 12 gate chunks
RZC = 8                    # r,z gate chunks (1024/128)
NC4 = 4                    # n-gate chunks (512/128)
ACH = A // 2               # 100: a-chunk size (2 chunks)
PC = H // 128              # 4 proj chunks
VCH = 125                  # v-chunk size (8 chunks of 125)
NVC = V // VCH             # 8

F32 = mybir.dt.float32
BF16 = mybir.dt.bfloat16
NP_BF16 = mybir.dt.np(BF16)

S_STEPS = int(os.environ.get("DEC_STEPS", S))
Act = mybir.ActivationFunctionType


# ---------------------------------------------------------------------------
# walrus workaround: CTRL-type instructions (Drain/Nop) only support one
# sem-wait each on this compiler; fan extra waits out onto preceding nops.
def _split_waits(nc, max_ctrl=1, max_other=1):
    emap = {
        mybir.EngineType.PE: nc.tensor,
        mybir.EngineType.DVE: nc.vector,
        mybir.EngineType.Activation: nc.scalar,
        mybir.EngineType.Pool: nc.gpsimd,
        mybir.EngineType.SP: nc.sync,
    }
    fn = nc.m.functions[0]
    tail_bb = fn.blocks[-1]
    for bb in list(fn.blocks):
        insts = bb.instructions
        i = 0
        while i < len(insts):
            inst = insts[i]
            si = inst.sync_info
            if si is None:
                i += 1
                continue
            lim = (max_ctrl if type(inst).__name__ in ("InstDrain", "InstNop")
                   else max_other)
            w = list(si.on_wait)
            if len(w) > lim:
                extra, keep = w[:-lim], w[-lim:]
                si.on_wait = keep
                pos = i
                for j in range(0, len(extra), max_ctrl):
                    chunk = extra[j : j + max_ctrl]
                    emap[inst.engine].nop(hint="waitsplit", nofuse=True)
                    ni = tail_bb.instructions.pop()
                    ni.sync_info = mybir.SyncInfo(on_wait=chunk, on_update=[])
                    insts.insert(pos, ni)
                    pos += 1
                    i += 1
            i += 1


def _bcast(ap, n_rep, pos):
    """Insert a stride-0 (broadcast) dim of size n_rep at free position pos."""
    lst = [list(x) for x in ap.ap]
    lst.insert(pos, [0, n_rep])
    return bass.AP(tensor=ap.tensor, offset=ap.offset, ap=lst)


# ---------------------------------------------------------------------------
def build_program(n_steps=S_STEPS):
    nc = bass.Bass("TRN2", target_bir_lowering=False, debug=False,
                   num_devices=NCORES)

    def din(name, shape, dt=BF16):
        return nc.dram_tensor(name, list(shape), dt, kind="ExternalInput")

    # --- DRAM inputs (per-core shards, host-marshalled layouts) ---
    encT = din("encT", (T, L, NSH, H))               # context stationary
    encH = din("encH", (128, HC, L, NSH, T))         # kproj moving
    kwT = din("kwT", (128, HC, L, A))                # kproj stationary
    qwT = din("qwT", (128, HC, L, A))                # qproj stationary
    kqb = din("kqb", (ACH, 2, L), F32)               # Kb+Qb, a-layout
    vw = din("vw", (ACH, 2, L), BF16)                # Vw columns
    vbB = din("vbB", (T, L), F32)                    # Vb broadcast
    wc0T = din("wc0T", (128, HC, G3))                # Wih0[:,E:].T
    whh0T = din("whh0T", (128, HC, G3))
    wih1T = din("wih1T", (128, HC, G3))
    whh1T = din("whh1T", (128, HC, G3))
    wi0eT = din("wi0eT", (128, EC, G3))              # Wih0[:,:E].T
    embvT = din("embvT", (128, EC, n_steps, NSH))    # gathered emb rows
    b0fold = din("b0fold", (128, GC), F32)           # bih0 + bhh0[rz] by gc
    bhh0nB = din("bhh0nB", (128, NC4, NSH), F32)     # bhh0[n] broadcast
    b1rzB = din("b1rzB", (128, RZC, NSH), F32)       # bih1+bhh1 rz broadcast
    b1inB = din("b1inB", (128, NC4, NSH), F32)       # bih1[n] broadcast
    bhh1nB = din("bhh1nB", (128, NC4, NSH), F32)     # bhh1[n] broadcast
    h0i = din("h0i", (128, HC, NSH), F32)            # initial states
    h1i = din("h1i", (128, HC, NSH), F32)
    pw0T = din("pw0T", (128, HC, H))
    pb0 = din("pb0", (128, PC), F32)
    pw1T = din("pw1T", (128, PC, V))
    pb1T = din("pb1T", (VCH, NVC), F32)

    out = nc.dram_tensor("out", [VCH, NVC, n_steps, NSH], F32,
                         kind="ExternalOutput")

    with tile.TileContext(nc) as tc, contextlib.ExitStack() as ctx:
        cst = ctx.enter_context(tc.tile_pool(name="cst", bufs=1))
        work = ctx.enter_context(tc.tile_pool(name="work", bufs=2))

        def load(pool, dram, tag):
            t = pool.tile(list(dram.shape), dram.dtype, tag=tag, name=tag)
            nc.sync.dma_start(out=t[:], in_=dram[:])
            return t

        # ---- resident constants ----
        encT_s = load(cst, encT, "encT")
        kqb_s = load(cst, kqb, "kqb")
        vw_s = load(cst, vw, "vw")
        vb_s = load(cst, vbB, "vbB")
        wc0_s = load(cst, wc0T, "wc0T")
        whh0_s = load(cst, whh0T, "whh0T")
        wih1_s = load(cst, wih1T, "wih1T")
        whh1_s = load(cst, whh1T, "whh1T")
        qw_s = load(cst, qwT, "qwT")
        b0f_s = load(cst, b0fold, "b0fold")
        bhh0n_s = load(cst, bhh0nB, "bhh0nB")
        b1rz_s = load(cst, b1rzB, "b1rzB")
        b1in_s = load(cst, b1inB, "b1inB")
        bhh1n_s = load(cst, bhh1nB, "bhh1nB")
        pb0_s = load(cst, pb0, "pb0")
        pb1_s = load(cst, pb1T, "pb1T")

        ones_col = cst.tile([128, 1], F32, tag="ones_col")
        nc.vector.memset(ones_col[:], 1.0)
        ones_row = cst.tile([1, 128], F32, tag="ones_row")
        nc.vector.memset(ones_row[:], 1.0)

        kproj = [cst.tile([ACH, L, NSH, T], F32, tag=f"kproj{a}",
                          name=f"kproj{a}")
                 for a in range(2)]
        gi_emb = cst.tile([128, GC, n_steps, NSH], F32, tag="gi_emb")
        h0a = cst.tile([128, HC, n_steps + 1, NSH], F32, tag="h0a")
        h1a = cst.tile([128, HC, n_steps + 1, NSH], F32, tag="h1a")

        # =================== init: Kproj / gi_emb / states ===================
        with tc.tile_pool(name="initsb", bufs=1) as isb, \
             tc.tile_pool(name="initps", bufs=1, space="PSUM") as ips:
            encH_s = load(isb, encH, "encH")
            kw_s = load(isb, kwT, "kwT")
            for ach in range(2):
                pk = ips.tile([ACH, L, NSH, T], F32, tag="pk")
                for l in range(L):
                    for nh in range(2):
                        for hc in range(HC):
                            nc.tensor.matmul(
                                pk[:, l, nh * 4:(nh + 1) * 4, :],
                                lhsT=kw_s[:, hc, l, ach * ACH:(ach + 1) * ACH],
                                rhs=encH_s[:, hc, l, nh * 4:(nh + 1) * 4, :],
                                start=(hc == 0), stop=(hc == HC - 1))
                    nc.scalar.activation(
                        out=kproj[ach][:, l, :, :], in_=pk[:, l, :, :],
                        func=Act.Copy, bias=0.0, scale=1.0)
                    nc.vector.tensor_scalar_add(
                        kproj[ach][:, l, :, :], kproj[ach][:, l, :, :],
                        kqb_s[:, ach, l:l + 1])

            wi0e_s = load(isb, wi0eT, "wi0eT")
            embv_s = load(isb, embvT, "embvT")
            embr = isb.tile([128, EC, n_steps, NSH], BF16, tag="embr")
            nc.scalar.activation(out=embr[:], in_=embv_s[:], func=Act.Relu,
                                 bias=0.0, scale=1.0)
            for gc in range(GC):
                pg = ips.tile([128, n_steps, NSH], F32, tag="pg")
                for ec in range(EC):
                    nc.tensor.matmul(
                        pg[:], lhsT=wi0e_s[:, ec, gc * 128:(gc + 1) * 128],
                        rhs=embr[:, ec, :, :],
                        start=(ec == 0), stop=(ec == EC - 1))
                nc.scalar.activation(out=gi_emb[:, gc, :, :], in_=pg[:],
                                     func=Act.Copy, bias=0.0, scale=1.0)
                nc.vector.tensor_scalar_add(
                    gi_emb[:, gc, :, :], gi_emb[:, gc, :, :],
                    b0f_s[:, gc:gc + 1])

            h0i_s = load(isb, h0i, "h0i")
            h1i_s = load(isb, h1i, "h1i")
            nc.vector.tensor_copy(h0a[:, :, 0, :], h0i_s[:])
            nc.vector.tensor_copy(h1a[:, :, 0, :], h1i_s[:])
            h0bf = work.tile([128, HC, NSH], BF16, tag="h0bf")
            h1bf = work.tile([128, HC, NSH], BF16, tag="h1bf")
            nc.vector.tensor_copy(h0bf[:], h0i_s[:])
            nc.vector.tensor_copy(h1bf[:], h1i_s[:])

        # =================== decode steps ===================
        with tc.tile_pool(name="psA", bufs=1, space="PSUM") as psA, \
             tc.tile_pool(name="psG", bufs=1, space="PSUM") as psG:

            def gru(layer, s, xin_bf, hprev_bf, wiT, whT, ha, brz, bin_, bhn):
                prz = psG.tile([128, RZC, NSH], F32, tag="prz")
                pin = psG.tile([128, NC4, NSH], F32, tag="pin")
                phn = psG.tile([128, NC4, NSH], F32, tag="phn")
                for gc in range(GC):
                    if gc < RZC:
                        dst = prz[:, gc, :]
                        for hc in range(HC):
                            nc.tensor.matmul(
                                dst, lhsT=wiT[:, hc, gc * 128:(gc + 1) * 128],
                                rhs=xin_bf[:, hc, :],
                                start=(hc == 0), stop=False)
                        for hc in range(HC):
                            nc.tensor.matmul(
                                dst, lhsT=whT[:, hc, gc * 128:(gc + 1) * 128],
                                rhs=hprev_bf[:, hc, :],
                                start=False, stop=(hc == HC - 1))
                    else:
                        g = gc - RZC
                        for hc in range(HC):
                            nc.tensor.matmul(
                                pin[:, g, :],
                                lhsT=wiT[:, hc, gc * 128:(gc + 1) * 128],
                                rhs=xin_bf[:, hc, :],
                                start=(hc == 0), stop=(hc == HC - 1))
                        for hc in range(HC):
                            nc.tensor.matmul(
                                phn[:, g, :],
                                lhsT=whT[:, hc, gc * 128:(gc + 1) * 128],
                                rhs=hprev_bf[:, hc, :],
                                start=(hc == 0), stop=(hc == HC - 1))
                rzs = work.tile([128, RZC, NSH], F32, tag="rzs")
                nc.vector.tensor_add(rzs[:], prz[:], brz)
                rz = work.tile([128, RZC, NSH], F32, tag="rz")
                nc.scalar.activation(out=rz[:], in_=rzs[:], func=Act.Sigmoid,
                                     bias=0.0, scale=1.0)
                hnb = work.tile([128, NC4, NSH], F32, tag="hnb")
                nc.vector.tensor_add(hnb[:], phn[:], bhn)
                rhn = work.tile([128, NC4, NSH], F32, tag="rhn")
                nc.vector.tensor_mul(rhn[:], rz[:, 0:NC4, :], hnb[:])
                na = work.tile([128, NC4, NSH], F32, tag="na")
                nc.vector.tensor_add(na[:], pin[:], bin_)
                na2 = work.tile([128, NC4, NSH], F32, tag="na2")
                nc.vector.tensor_add(na2[:], na[:], rhn[:])
                nt = work.tile([128, NC4, NSH], F32, tag="nt")
                nc.scalar.activation(out=nt[:], in_=na2[:], func=Act.Tanh,
                                     bias=0.0, scale=1.0)
                d = work.tile([128, NC4, NSH], F32, tag="d")
                nc.vector.tensor_sub(d[:], ha[:, :, s, :], nt[:])
                zd = work.tile([128, NC4, NSH], F32, tag="zd")
                nc.vector.tensor_mul(zd[:], rz[:, NC4:RZC, :], d[:])
                nc.vector.tensor_add(ha[:, :, s + 1, :], nt[:], zd[:])
                hbf_new = work.tile([128, HC, NSH], BF16, tag=f"hbf{layer}")
                nc.vector.tensor_copy(hbf_new[:], ha[:, :, s + 1, :])
                return hbf_new

            for s in range(n_steps):
                hbf = [h0bf, h1bf]
                # --- qproj: pq[a, ach, l, n] ---
                pq = psA.tile([ACH, 2, L, NSH], F32, tag="pq")
                for ach in range(2):
                    for l in range(L):
                        for hc in range(HC):
                            nc.tensor.matmul(
                                pq[:, ach, l, :],
                                lhsT=qw_s[:, hc, l, ach * ACH:(ach + 1) * ACH],
                                rhs=hbf[l][:, hc, :],
                                start=(hc == 0), stop=(hc == HC - 1))
                # --- e = tanh(q + kproj), bf16, layout [a, ach, l, n, t] ---
                ebf = work.tile([ACH, 2, L, NSH, T], BF16, tag="ebf", bufs=1)
                for ach in range(2):
                    earg = work.tile([ACH, L, NSH, T], F32, tag="earg", bufs=1)
                    qb = _bcast(pq[:, ach, :, :], T, 3)
                    nc.vector.tensor_add(earg[:], kproj[ach][:], qb)
                    nc.scalar.activation(out=ebf[:, ach, :, :, :], in_=earg[:],
                                         func=Act.Tanh, bias=0.0, scale=1.0)
                # --- scoresT [t, l, n] via e-stationary matvecs ---
                psc = psA.tile([T, L, NSH], F32, tag="psc")
                for l in range(L):
                    for n in range(NSH):
                        for ach in range(2):
                            nc.tensor.matmul(
                                psc[:, l, n:n + 1],
                                lhsT=ebf[:, ach, l, n, :],
                                rhs=vw_s[:, ach, l:l + 1],
                                start=(ach == 0), stop=(ach == 1))
                # --- softmax over (l, t) per case (no max-sub; scores bounded) ---
                u = work.tile([T, L, NSH], F32, tag="u")
                for l in range(L):
                    nc.scalar.activation(out=u[:, l, :], in_=psc[:, l, :],
                                         func=Act.Exp,
                                         bias=vb_s[:, l:l + 1], scale=1.0)
                ubf = work.tile([T, L, NSH], BF16, tag="ubf")
                nc.vector.tensor_copy(ubf[:], u[:])
                pu = psA.tile([1, L, NSH], F32, tag="pu")
                nc.tensor.matmul(pu[:, 0, :], lhsT=ones_col[:],
                                 rhs=u[:, 0, :], start=True, stop=False)
                nc.tensor.matmul(pu[:, 0, :], lhsT=ones_col[:],
                                 rhs=u[:, 1, :], start=False, stop=True)
                ssum = work.tile([1, NSH], F32, tag="ssum")
                nc.scalar.activation(out=ssum[:], in_=pu[:, 0, :],
                                     func=Act.Copy, bias=0.0, scale=1.0)
                rs = work.tile([1, NSH], F32, tag="rs")
                nc.vector.reciprocal(rs[:], ssum[:])
                prs = psA.tile([128, NSH], F32, tag="prs")
                nc.tensor.matmul(prs[:], lhsT=ones_row[:], rhs=rs[:],
                                 start=True, stop=True)
                rsr = work.tile([128, NSH], F32, tag="rsr")
                nc.scalar.activation(out=rsr[:], in_=prs[:], func=Act.Copy,
                                     bias=0.0, scale=1.0)
                # --- context: ctxT[h, n] = (sum_lt u * encT) / s ---
                pctx = psA.tile([128, HC, NSH], F32, tag="pctx")
                for n in range(NSH):
                    for hc in range(HC):
                        for l in range(L):
                            nc.tensor.matmul(
                                pctx[:, hc, n:n + 1],
                                lhsT=encT_s[:, l, n, hc * 128:(hc + 1) * 128],
                                rhs=ubf[:, l, n:n + 1],
                                start=(l == 0), stop=(l == 1))
                ctxbf = work.tile([128, HC, NSH], BF16, tag="ctxbf")
                nc.vector.tensor_mul(ctxbf[:], pctx[:], _bcast(rsr[:], HC, 1))

                h0bf = gru(0, s, ctxbf, h0bf, wc0_s, whh0_s, h0a,
                           gi_emb[:, 0:RZC, s, :], gi_emb[:, RZC:GC, s, :],
                           bhh0n_s[:])
                h1bf = gru(1, s, h0bf, h1bf, wih1_s, whh1_s, h1a,
                           b1rz_s[:], b1in_s[:], bhh1n_s[:])

        # =================== projection + logits ===================
        with tc.tile_pool(name="postsb", bufs=1) as osb, \
             tc.tile_pool(name="postps", bufs=2, space="PSUM") as ops:
            pw0_s = load(osb, pw0T, "pw0T")
            pw1_s = load(osb, pw1T, "pw1T")
            h1allbf = osb.tile([128, HC, n_steps, NSH], BF16, tag="h1allbf")
            nc.vector.tensor_copy(h1allbf[:], h1a[:, :, 1:n_steps + 1, :])
            projbf = osb.tile([128, PC, n_steps, NSH], BF16, tag="projbf")
            for pc in range(PC):
                pp = ops.tile([128, n_steps, NSH], F32, tag="pp")
                for hc in range(HC):
                    nc.tensor.matmul(
                        pp[:], lhsT=pw0_s[:, hc, pc * 128:(pc + 1) * 128],
                        rhs=h1allbf[:, hc, :, :],
                        start=(hc == 0), stop=(hc == HC - 1))
                nc.scalar.activation(out=projbf[:, pc, :, :], in_=pp[:],
                                     func=Act.Relu, bias=pb0_s[:, pc:pc + 1],
                                     scale=1.0)
            lg = osb.tile([VCH, NVC, n_steps, NSH], F32, tag="lg")
            for vc in range(NVC):
                pl = ops.tile([VCH, n_steps, NSH], F32, tag="pl")
                for pc in range(PC):
                    nc.tensor.matmul(
                        pl[:], lhsT=pw1_s[:, pc, vc * VCH:(vc + 1) * VCH],
                        rhs=projbf[:, pc, :, :],
                        start=(pc == 0), stop=(pc == PC - 1))
                nc.scalar.activation(out=lg[:, vc, :, :], in_=pl[:],
                                     func=Act.Copy, bias=0.0, scale=1.0)
                nc.vector.tensor_scalar_add(lg[:, vc, :, :], lg[:, vc, :, :],
                                            pb1_s[:, vc:vc + 1])
            nc.sync.dma_start(out=out[:], in_=lg[:])

    _split_waits(nc)
    return nc


# ---------------------------------------------------------------------------
def prep_core_inputs(inputs, core, n_steps=S_STEPS):
    """Marshal full inputs -> per-core DRAM tensors (layout/dtype only)."""
    f32 = np.float32
    sl = slice(core * NSH, (core + 1) * NSH)
    enc = np.asarray(inputs["encoder_outputs"], f32)[:, :, sl, :]  # L,T,n,H
    fin = np.asarray(inputs["encoder_final_states"], f32)[:, sl, :]
    tgt = np.asarray(inputs["targets"])[sl, :]

    d = {}
    d["encT"] = np.ascontiguousarray(
        enc.transpose(1, 0, 2, 3)).astype(NP_BF16)          # T,L,n,H
    eh = enc.transpose(3, 0, 2, 1).reshape(HC, 128, L, NSH, T)
    d["encH"] = np.ascontiguousarray(
        eh.transpose(1, 0, 2, 3, 4)).astype(NP_BF16)        # 128,HC,L,n,T

    def wT_hlayout(w):  # w: [out_dim, H] -> [128, HC, out_dim]
        x = w.T.reshape(HC, 128, -1)
        return np.ascontiguousarray(x.transpose(1, 0, 2))

    kw = np.asarray(inputs["Kw"], f32)   # L,A,H
    qw = np.asarray(inputs["Qw"], f32)
    d["kwT"] = np.stack(
        [wT_hlayout(kw[l]) for l in range(L)], axis=2).astype(NP_BF16)
    d["qwT"] = np.stack(
        [wT_hlayout(qw[l]) for l in range(L)], axis=2).astype(NP_BF16)
    kqbias = np.asarray(inputs["Kb"], f32) + np.asarray(inputs["Qb"], f32)
    d["kqb"] = np.ascontiguousarray(
        kqbias.T.reshape(2, ACH, L).transpose(1, 0, 2)).astype(f32)
    vwv = np.asarray(inputs["Vw"], f32)[:, 0, :]  # L,A
    d["vw"] = np.ascontiguousarray(
        vwv.T.reshape(2, ACH, L).transpose(1, 0, 2)).astype(NP_BF16)
    vb = np.asarray(inputs["Vb"], f32)[:, 0]      # L
    d["vbB"] = np.ascontiguousarray(
        np.broadcast_to(vb[None, :], (T, L))).astype(f32)

    wih0 = np.asarray(inputs["Wih0"], f32)
    d["wc0T"] = wT_hlayout(wih0[:, E:]).astype(NP_BF16)
    d["whh0T"] = wT_hlayout(np.asarray(inputs["Whh0"], f32)).astype(NP_BF16)
    d["wih1T"] = wT_hlayout(np.asarray(inputs["Wih1"], f32)).astype(NP_BF16)
    d["whh1T"] = wT_hlayout(np.asarray(inputs["Whh1"], f32)).astype(NP_BF16)
    wi0e = wih0[:, :E].T.reshape(EC, 128, G3)
    d["wi0eT"] = np.ascontiguousarray(wi0e.transpose(1, 0, 2)).astype(NP_BF16)

    toks = np.concatenate(
        [np.full((NSH, 1), SOS_ID, tgt.dtype), tgt[:, :-1]], axis=1)
    toks = np.asarray(toks[:, :n_steps])
    embW = np.asarray(inputs["emb_W"], f32)
    ev = embW[toks]                                  # n, s, E
    ev = ev.transpose(2, 1, 0).reshape(EC, 128, n_steps, NSH)
    d["embvT"] = np.ascontiguousarray(ev.transpose(1, 0, 2, 3)).astype(NP_BF16)

    bih0 = np.asarray(inputs["bih0"], f32)
    bhh0 = np.asarray(inputs["bhh0"], f32)
    bih1 = np.asarray(inputs["bih1"], f32)
    bhh1 = np.asarray(inputs["bhh1"], f32)
    b0 = bih0.copy()
    b0[:1024] += bhh0[:1024]
    d["b0fold"] = np.ascontiguousarray(b0.reshape(GC, 128).T).astype(f32)

    def bc(v, nchunks):
        x = v.reshape(nchunks, 128).T[:, :, None]
        return np.ascontiguousarray(
            np.broadcast_to(x, (128, nchunks, NSH))).astype(f32)

    d["bhh0nB"] = bc(bhh0[1024:], NC4)
    d["b1rzB"] = bc((bih1 + bhh1)[:1024], RZC)
    d["b1inB"] = bc(bih1[1024:], NC4)
    d["bhh1nB"] = bc(bhh1[1024:], NC4)

    def hlay(x):  # [n, H] -> [128, HC, n]
        return np.ascontiguousarray(
            x.T.reshape(HC, 128, NSH).transpose(1, 0, 2)).astype(f32)

    d["h0i"] = hlay(fin[0])
    d["h1i"] = hlay(fin[1])

    d["pw0T"] = wT_hlayout(np.asarray(inputs["Pw0"], f32)).astype(NP_BF16)
    d["pb0"] = np.ascontiguousarray(
        np.asarray(inputs["Pb0"], f32).reshape(PC, 128).T).astype(f32)
    d["pw1T"] = wT_hlayout(np.asarray(inputs["Pw1"], f32)).astype(NP_BF16)
    d["pb1T"] = np.ascontiguousarray(
        np.asarray(inputs["Pb1"], f32).reshape(NVC, VCH).T).astype(f32)
    return d


_PROG_CACHE = {}
_LAST_RESULT = {}


def kernel(**inputs):
    n_steps = S_STEPS
    if n_steps not in _PROG_CACHE:
        _PROG_CACHE[n_steps] = build_program(n_steps)
    nc = _PROG_CACHE[n_steps]
    in_maps = [prep_core_inputs(inputs, c, n_steps) for c in range(NCORES)]
    trace = os.environ.get("DEC_TRACE") == "1"
    res = run_bass_kernel_spmd(nc, in_maps, list(range(NCORES)), trace=trace)
    _LAST_RESULT["res"] = res
    outs = []
    for c in range(NCORES):
        o = np.asarray(res.results[c]["out"], np.float32)  # VCH,NVC,S,NSH
        o = o.transpose(3, 2, 1, 0).reshape(NSH, n_steps, V)
        outs.append(o)
    return np.concatenate(outs, axis=0).astype(np.float32)
